# revision 6
# baseline (speedup 1.0000x reference)
"""Trainium2 Bass kernel for nn_FEDformerEncoder (8-core data parallel).

The reference network is, per layer (L=2):
    y  = mean_e( conv1d_same(x, w_e) + b_e )              (depthwise conv on W)
    q,k,v = y @ w{q,k,v}.T + b{q,k,v}                     ([rows, P])
    Q,K,V = fft(q),fft(k),fft(v)
    Wt = K * conj(Q) / sqrt(P) * V
    out = ifft(Wt).real @ wo.T + bo

Everything except the elementwise complex triple product is linear in x, so
the conv, the FFT, and the iFFT fold into host-precomputed projection
weights; the whole network collapses into three matmul stages and two
elementwise stages (see _fold_layer).  Weights are repacked host-side into
a handful of large consumption-ordered DRAM tensors so the kernel issues
few, big DMAs that land exactly in matmul order (many small DMAs serialize
on the shared HWDGE completion-semaphore lanes and starve the PE).
"""
import sys

import numpy as np

sys.path.insert(0, "/opt/trn_rl_repo")

import concourse.bass as bass
import concourse.mybir as mybir
import concourse.tile as tile
from concourse import bacc
from concourse.bass_utils import run_bass_kernel_spmd

BS, CNT, W, P, E, KK, L = 32, 128, 2048, 1024, 8, 25, 2
H = P // 2                    # 512 slots per packed block
NCORES = 8
ROWS = (BS // NCORES) * CNT   # 512 rows per core
KT = W // 128                 # 16 contraction tiles (stage 1)
MT = (3 * P) // 128           # 24 output tiles (stages 1,2: q|k|v packed)
ST = P // 128                 # 8 contraction tiles (stages 2,3)
WT = W // 128                 # 16 output tiles (stage 3)
F32 = mybir.dt.float32
ACT = mybir.dt.float16
ACT_NP = np.float16
IDENT = mybir.ActivationFunctionType.Identity


def _fold_layer(conv_w, conv_b, wq, bq, wk, bk, wv, bv, wo, bo):
    """Fold conv + FFT into projection weights (float64 math).

    Returns EW [W, 3*P], Sbias [3*P], WoP [P, W] (rows interleaved
    [A0 B0 A1 B1 A2 B2 A3 B3] by 128-tile), bo [W].
    """
    f64 = np.float64
    wbar = conv_w.astype(f64).mean(axis=0)[0]          # [KK]
    bbar = conv_b.astype(f64).mean()

    idx = np.arange(W)
    D = idx[None, :] - idx[:, None] + (KK // 2)        # C[w,u] = wbar[u-w+12]
    C = np.where((D >= 0) & (D < KK), wbar[np.clip(D, 0, KK - 1)], 0.0)

    def pack_fwd(wmat, bvec, scale=1.0):
        Wf = np.fft.fft(wmat.astype(f64), axis=0)      # [P, W]
        Bf = np.fft.fft(bvec.astype(f64))              # [P]
        cols = np.empty((W, P), dtype=f64)
        cols[:, :H] = Wf[:H, :].real.T
        cols[:, H] = Wf[H, :].real
        cols[:, H + 1:] = Wf[1:H, :].imag.T
        bias = np.empty(P, dtype=f64)
        bias[:H] = Bf[:H].real
        bias[H] = Bf[H].real
        bias[H + 1:] = Bf[1:H].imag
        return cols * scale, bias * scale

    s = 1.0 / np.sqrt(f64(P))
    cq, bq_p = pack_fwd(wq, bq)
    ck, bk_p = pack_fwd(wk, bk)
    cv, bv_p = pack_fwd(wv, bv, scale=s)
    cols = np.concatenate([cq, ck, cv], axis=1)        # [W, 3P]
    bias = np.concatenate([bq_p, bk_p, bv_p])

    EW = C.T @ cols
    Sbias = bbar * cols.sum(axis=0) + bias

    G = np.fft.ifft(wo.astype(f64), axis=1)            # [W, P]
    WoP = np.empty((P, W), dtype=f64)
    WoP[0] = G[:, 0].real
    WoP[1:H] = 2.0 * G[:, 1:H].real.T
    WoP[H] = G[:, H].real
    WoP[H + 1:] = -2.0 * G[:, 1:H].imag.T
    # interleave row-tiles A0 B0 A1 B1 ... to match Wcat production order
    WoP = WoP.reshape(2, 4, 128, W).transpose(1, 0, 2, 3).reshape(P, W)

    return EW, Sbias, WoP, bo.astype(f64)


def _build_module():
    nc = bacc.Bacc("TRN2", target_bir_lowering=False, debug=False)

    # x: one [128, KT*ROWS] tensor; column block k = contraction k-tile.
    xin = nc.dram_tensor("xin", [128, KT * ROWS], ACT, kind="ExternalInput")
    # stage-1 t=0 weights: all six chains k-interleaved: col (k*6+b)*128
    ewt0 = nc.dram_tensor("ewt0", [128, 6 * W], ACT, kind="ExternalInput")
    # stage-1 groups t=1..3: 6 chain-tiles each, chain-major
    ew1g = nc.dram_tensor("ew1g", [3, 128, 6 * W], ACT, kind="ExternalInput")
    # stage-2 groups t=0..3: 6 chain-tiles each, chain-major
    m12g = nc.dram_tensor("m12g", [4, 128, 6 * P], ACT, kind="ExternalInput")
    # stage-3 weights in quarters of 4 output tiles
    wop2q = nc.dram_tensor("wop2q", [4, 128, 4 * P], ACT,
                           kind="ExternalInput")
    # [sb1(24) | sb2(24) | bo2(16)] packed into one [128, 64] tensor
    biases = nc.dram_tensor("biases", [128, 2 * MT + WT], F32,
                            kind="ExternalInput")
    xout = nc.dram_tensor("xout", [WT, 128, ROWS], ACT, kind="ExternalOutput")

    with tile.TileContext(nc) as tc:
        with (
            tc.tile_pool(name="xbuf", bufs=1) as xpool,
            tc.tile_pool(name="bias", bufs=1) as bpool,
            tc.tile_pool(name="warm", bufs=1) as warmpool,
            tc.tile_pool(name="wt0", bufs=1) as wt0pool,
            tc.tile_pool(name="wfg", bufs=2) as wfgpool,
            tc.tile_pool(name="wm", bufs=2) as wmpool,
            tc.tile_pool(name="wo2", bufs=4) as wo2pool,
            tc.tile_pool(name="spec", bufs=16) as spool,
            tc.tile_pool(name="wt", bufs=16) as wtpool,
            tc.tile_pool(name="ew", bufs=8) as ewpool,
            tc.tile_pool(name="out", bufs=4) as opool,
            tc.tile_pool(name="psum", bufs=8, space="PSUM") as pspool,
        ):
            # ---- PE warm-up: the HAM clock gate holds the PE at 1.2 GHz
            # until it has seen ~3.4us of gap-free activity.  These dummies
            # run while the first DMAs are in flight, timed to end as the
            # first real operands land, so real matmuls start at 2.4 GHz.
            wdummy = warmpool.tile([128, 256], ACT, tag="warm")
            nc.vector.memset(wdummy[:], 0.0)
            psd = pspool.tile([128, 256], F32, tag="ps", name="psd")
            for _ in range(14):
                nc.tensor.matmul(psd[:], wdummy[:, 0:128], wdummy[:],
                                 start=True, stop=True)

            # ---- input streaming: few, large, need-ordered DMAs.
            # sync carries the t=0 weight mega-tile (k-order chunks so the
            # first matmul waits only for one 192KB chunk), scalar carries
            # x (first chunk halved so k=0 lands sooner).  All later weight
            # groups ride the otherwise-idle gpsimd queue, so they stream
            # concurrently instead of queueing behind wt0 on sync (the
            # baseline stalled ~0.9us at the t=0 -> t=1 boundary waiting
            # for ew1g[0] to drain through sync).  Chunks stay coarse: many
            # small DMAs serialize on the shared HWDGE completion-semaphore
            # lanes and starve the PE.
            wt0 = wt0pool.tile([128, 6 * W], ACT, tag="wt0")
            xb = xpool.tile([128, KT * ROWS], ACT, tag="x")
            for lo, hi in ((0, 768), (768, 1536), (1536, 3072),
                           (3072, 6144), (6144, 9216), (9216, 12288)):
                nc.sync.dma_start(wt0[:, lo:hi], ewt0[:, lo:hi])

            for lo, hi in ((0, 256), (256, 512), (512, 1024), (1024, 1536),
                           (1536, 2048), (2048, 3072), (3072, 4096),
                           (4096, 6144), (6144, 8192)):
                nc.scalar.dma_start(xb[:, lo:hi], xin[:, lo:hi])

            btile = bpool.tile([128, 2 * MT + WT], F32, tag="biases")
            nc.sync.dma_start(btile[:], biases[:])

            def xslice(k):
                return xb[:, bass.ts(k, ROWS)]

            def sb1_col(j):
                return btile[:, j:j + 1]

            def sb2_col(j):
                return btile[:, MT + j:MT + j + 1]

            def bo2_col(j):
                return btile[:, 2 * MT + j:2 * MT + j + 1]

            def elementwise(St, first):
                """complex triple product on one partition-row group.

                All operands fp16: contiguous 16-bit tensor_tensor hits the
                DVE's 2x packed mode, halving vector time.
                """
                qA, qB, kA, kB, vA, vB = St
                v = nc.vector
                cr = ewpool.tile([128, ROWS], ACT, tag="ew")
                ci = ewpool.tile([128, ROWS], ACT, tag="ew")
                t0 = ewpool.tile([128, ROWS], ACT, tag="ew")
                v.tensor_mul(cr[:], kA[:], qA[:])
                v.tensor_mul(t0[:], kB[:], qB[:])
                v.tensor_add(cr[:], cr[:], t0[:])
                v.tensor_mul(ci[:], kB[:], qA[:])
                v.tensor_mul(t0[:], kA[:], qB[:])
                v.tensor_sub(ci[:], ci[:], t0[:])
                wr = wtpool.tile([128, ROWS], ACT, tag="wt")
                wi = wtpool.tile([128, ROWS], ACT, tag="wt")
                v.tensor_mul(wr[:], cr[:], vA[:])
                v.tensor_mul(t0[:], ci[:], vB[:])
                v.tensor_sub(wr[:], wr[:], t0[:])
                v.tensor_mul(wi[:], cr[:], vB[:])
                v.tensor_mul(t0[:], ci[:], vA[:])
                v.tensor_add(wi[:], wi[:], t0[:])
                if first:
                    # slot 0: A holds DC, B holds Nyquist — both real
                    v.tensor_mul(t0[0:1, :], qA[0:1, :], kA[0:1, :])
                    v.tensor_mul(wr[0:1, :], t0[0:1, :], vA[0:1, :])
                    v.tensor_mul(t0[0:1, :], qB[0:1, :], kB[0:1, :])
                    v.tensor_mul(wi[0:1, :], t0[0:1, :], vB[0:1, :])
                return wr, wi

            # ---- stage 1: S1 = x @ EW1 + b1, pipelined elementwise ----
            # t=0: chains b=0..3 are k-interleaved in both issue order and
            # the weight layout, so weights and x arrive exactly in matmul
            # order while the PE is still cold.
            Wcat1 = [None] * ST
            wfg_tiles = {}
            for t in range(4):
                if t < 3:                       # prefetch group t+1 weights
                    wfgt = wfgpool.tile([128, 6 * W], ACT, tag="wfg",
                                        name="wfg")
                    nc.gpsimd.dma_start(wfgt[:, 0:6144], ew1g[t][:, 0:6144])
                    nc.gpsimd.dma_start(wfgt[:, 6144:], ew1g[t][:, 6144:])
                    wfg_tiles[t + 1] = wfgt
                if t == 3:                      # prefetch first m12 group
                    wm0 = wmpool.tile([128, 6 * P], ACT, tag="wm", name="wm")
                    nc.gpsimd.dma_start(wm0[:, 0:3072], m12g[0][:, 0:3072])
                    nc.gpsimd.dma_start(wm0[:, 3072:], m12g[0][:, 3072:])
                    wfg_tiles["m0"] = wm0

                St = []
                if t == 0:
                    # all six chains k-interleaved: weights and x are
                    # consumed in exactly DMA-arrival order, and the tiny
                    # data-independent fillers keep the HAM activity window
                    # busy through any arrival jitter while the PE warms.
                    chains = [pspool.tile([128, ROWS], F32, tag="ps",
                                          name="ps") for _ in range(6)]
                    for k in range(KT):
                        for b in range(6):
                            nc.tensor.matmul(
                                chains[b][:],
                                wt0[:, bass.ts(k * 6 + b, 128)], xslice(k),
                                start=(k == 0), stop=(k == KT - 1))
                    for b in range(6):
                        Sj = spool.tile([128, ROWS], ACT, tag="spec")
                        nc.scalar.activation(Sj[:], chains[b][:], IDENT,
                                             bias=sb1_col(b * 4))
                        St.append(Sj)
                else:
                    wfgt = wfg_tiles[t]
                    for b in range(6):
                        j = b * 4 + t
                        ps = pspool.tile([128, ROWS], F32, tag="ps")
                        for k in range(KT):
                            nc.tensor.matmul(
                                ps[:], wfgt[:, b * W:][:, bass.ts(k, 128)],
                                xslice(k),
                                start=(k == 0), stop=(k == KT - 1))
                        Sj = spool.tile([128, ROWS], ACT, tag="spec")
                        nc.scalar.activation(Sj[:], ps[:], IDENT,
                                             bias=sb1_col(j))
                        St.append(Sj)
                wr, wi = elementwise(St, t == 0)
                Wcat1[2 * t] = wr
                Wcat1[2 * t + 1] = wi

            # ---- stage 2: S2 = Wt1 @ M12 + b2, pipelined elementwise ----
            # m12 group t+1 and the wop2 quarters prefetch during group t.
            Wcat2 = [None] * ST
            wo2_tiles = []
            for t in range(4):
                wmt = wfg_tiles.pop("m0") if t == 0 else wfg_tiles[("m", t)]
                if t < 3:
                    wmn = wmpool.tile([128, 6 * P], ACT, tag="wm", name="wm")
                    nc.gpsimd.dma_start(wmn[:, 0:3072], m12g[t + 1][:, 0:3072])
                    nc.gpsimd.dma_start(wmn[:, 3072:], m12g[t + 1][:, 3072:])
                    wfg_tiles[("m", t + 1)] = wmn
                if t < 4:                       # wop2 quarter prefetch
                    w2 = wo2pool.tile([128, 4 * P], ACT, tag="wo2",
                                      name="wo2")
                    nc.gpsimd.dma_start(w2[:], wop2q[t])
                    wo2_tiles.append(w2)

                St = []
                for b in range(6):
                    j = b * 4 + t
                    ps = pspool.tile([128, ROWS], F32, tag="ps")
                    for s in range(ST):
                        nc.tensor.matmul(
                            ps[:], wmt[:, b * P:][:, bass.ts(s, 128)],
                            Wcat1[s][:],
                            start=(s == 0), stop=(s == ST - 1))
                    Sj = spool.tile([128, ROWS], ACT, tag="spec")
                    nc.scalar.activation(Sj[:], ps[:], IDENT,
                                         bias=sb2_col(j))
                    St.append(Sj)
                wr, wi = elementwise(St, t == 0)
                Wcat2[2 * t] = wr
                Wcat2[2 * t + 1] = wi

            # ---- stage 3: out = Wt2 @ WoP2 + bo2 ----
            # evac+bias on the scalar engine; fp16 output; the final tile is
            # split in half so its evac/DMA tail is shorter.
            # first two tiles contract s=0..5 first (ready since mid-stage
            # 2) and finish with s=6,7 once the last elementwise group
            # lands, so the PE rolls through the 2->3 boundary stall-free
            wave = []
            for j in (0, 1):
                wcol = wo2_tiles[0][:, j * P:]
                ps = pspool.tile([128, ROWS], F32, tag="ps", name="ps")
                for s in range(ST - 2):
                    nc.tensor.matmul(
                        ps[:], wcol[:, bass.ts(s, 128)], Wcat2[s][:],
                        start=(s == 0), stop=False)
                wave.append((j, wcol, ps))
            for j, wcol, ps in wave:
                for s in (ST - 2, ST - 1):
                    nc.tensor.matmul(
                        ps[:], wcol[:, bass.ts(s, 128)], Wcat2[s][:],
                        start=False, stop=(s == ST - 1))
                ostage = opool.tile([128, ROWS], ACT, tag="out")
                nc.scalar.activation(ostage[:], ps[:], IDENT,
                                     bias=bo2_col(j))
                (nc.scalar if j % 2 else nc.sync).dma_start(
                    xout[j], ostage[:])
            for j in range(2, WT - 1):
                w2 = wo2_tiles[j // 4]
                wcol = w2[:, (j % 4) * P:]
                ps = pspool.tile([128, ROWS], F32, tag="ps")
                for s in range(ST):
                    nc.tensor.matmul(
                        ps[:], wcol[:, bass.ts(s, 128)], Wcat2[s][:],
                        start=(s == 0), stop=(s == ST - 1))
                ostage = opool.tile([128, ROWS], ACT, tag="out")
                nc.scalar.activation(ostage[:], ps[:], IDENT,
                                     bias=bo2_col(j))
                (nc.scalar if j % 2 else nc.sync).dma_start(
                    xout[j], ostage[:])

            # final tile: two half-row PSUM chains in separate banks so
            # the evacs run on scalar AND vector in parallel and both
            # output DMAs overlap — shortens the kernel-tail drain.
            jl = WT - 1
            wcol = wo2_tiles[jl // 4][:, (jl % 4) * P:]
            ps_a = pspool.tile([128, ROWS // 2], F32, tag="ps", name="psa")
            ps_b = pspool.tile([128, ROWS // 2], F32, tag="ps", name="psb")
            for s in range(ST):
                nc.tensor.matmul(
                    ps_a[:], wcol[:, bass.ts(s, 128)],
                    Wcat2[s][:, 0:ROWS // 2],
                    start=(s == 0), stop=(s == ST - 1))
            for s in range(ST):
                nc.tensor.matmul(
                    ps_b[:], wcol[:, bass.ts(s, 128)],
                    Wcat2[s][:, ROWS // 2:],
                    start=(s == 0), stop=(s == ST - 1))
            oa = opool.tile([128, ROWS // 2], ACT, tag="outh")
            nc.scalar.activation(oa[:], ps_a[:], IDENT, bias=bo2_col(jl))
            nc.sync.dma_start(xout[jl][:, 0:ROWS // 2], oa[:])
            ob = opool.tile([128, ROWS // 2], ACT, tag="outh")
            nc.vector.tensor_scalar_add(ob[:], ps_b[:], bo2_col(jl))
            nc.scalar.dma_start(xout[jl][:, ROWS // 2:], ob[:])
    nc.compile()
    return nc


_MODULE_CACHE = {}


def _get_module():
    if "nc" not in _MODULE_CACHE:
        _MODULE_CACHE["nc"] = _build_module()
    return _MODULE_CACHE["nc"]


def _prepare_weight_maps(conv_w, conv_b, wq, bq, wk, bk, wv, bv, wo, bo):
    folds = [_fold_layer(conv_w[l], conv_b[l], wq[l], bq[l], wk[l], bk[l],
                         wv[l], bv[l], wo[l], bo[l]) for l in range(L)]
    EW1, Sb1, WoP1, _bo1 = folds[0]
    EW2, Sb2, WoP2, bo2 = folds[1]
    M12 = WoP1 @ EW2                               # [P, 3P], fp64
    Sb2e = _bo1 @ EW2 + Sb2                        # [3P]

    def pack(Wm, n_k, n_m):
        # [n_k*128, n_m*128] -> [n_m, 128, n_k*128] partition-contiguous
        return np.ascontiguousarray(
            Wm.reshape(n_k, 128, n_m, 128).transpose(2, 1, 0, 3)
            .reshape(n_m, 128, n_k * 128).astype(ACT_NP))

    ew1 = pack(EW1, KT, MT)                        # [24, 128, 2048]
    m12 = pack(M12, ST, MT)                        # [24, 128, 1024]
    wop2 = pack(WoP2, ST, WT)                      # [16, 128, 1024]

    # t=0 mega-tile: all six chains k-interleaved, col ((k*6+b)*128 + m)
    ewt0 = (np.stack([ew1[b * 4] for b in range(6)])      # [6,128,2048]
            .reshape(6, 128, KT, 128).transpose(1, 2, 0, 3)
            .reshape(128, 6 * W))

    ew1g = np.stack([
        np.concatenate([ew1[b * 4 + t] for b in range(6)], axis=1)
        for t in (1, 2, 3)])
    m12gv = np.stack([
        np.concatenate([m12[b * 4 + t] for b in range(6)], axis=1)
        for t in range(4)])
    wop2qv = np.stack([
        np.concatenate([wop2[q * 4 + j] for j in range(4)], axis=1)
        for q in range(4)])

    biases = np.concatenate([
        Sb1.reshape(MT, 128).T, Sb2e.reshape(MT, 128).T,
        bo2.reshape(WT, 128).T], axis=1).astype(np.float32)
    return {
        "ewt0": np.ascontiguousarray(ewt0),
        "ew1g": np.ascontiguousarray(ew1g),
        "m12g": np.ascontiguousarray(m12gv),
        "wop2q": np.ascontiguousarray(wop2qv),
        "biases": np.ascontiguousarray(biases),
    }


def _make_in_maps(inputs):
    x = np.asarray(inputs["x"], dtype=np.float32)
    wmap = _prepare_weight_maps(
        np.asarray(inputs["conv_w"]), np.asarray(inputs["conv_b"]),
        np.asarray(inputs["wq"]), np.asarray(inputs["bq"]),
        np.asarray(inputs["wk"]), np.asarray(inputs["bk"]),
        np.asarray(inputs["wv"]), np.asarray(inputs["bv"]),
        np.asarray(inputs["wo"]), np.asarray(inputs["bo"]))
    per_core = BS // NCORES
    in_maps = []
    for c in range(NCORES):
        xc = x[c * per_core:(c + 1) * per_core].reshape(ROWS, W)
        xin = np.ascontiguousarray(
            xc.reshape(ROWS, KT, 128).transpose(2, 1, 0)
            .reshape(128, KT * ROWS).astype(ACT_NP))
        in_maps.append({"xin": xin, **wmap})
    return in_maps


def kernel(x, conv_w, conv_b, wq, bq, wk, bk, wv, bv, wo, bo):
    in_maps = _make_in_maps(dict(
        x=x, conv_w=conv_w, conv_b=conv_b, wq=wq, bq=bq, wk=wk, bk=bk,
        wv=wv, bv=bv, wo=wo, bo=bo))
    nc = _get_module()
    res = run_bass_kernel_spmd(nc, in_maps, list(range(NCORES)))

    per_core = BS // NCORES
    outs = []
    for c in range(NCORES):
        xo = res.results[c]["xout"]                    # [WT, 128, ROWS]
        outs.append(xo.astype(np.float32)
                    .transpose(2, 0, 1).reshape(per_core, CNT, W))
    return np.concatenate(outs, axis=0).astype(np.float32)



# revision 8
# speedup vs baseline: 1.1779x; 1.1779x over previous
"""Trainium2 Bass kernel for nn_FEDformerEncoder (8-core data parallel).

The reference network is, per layer (L=2):
    y  = mean_e( conv1d_same(x, w_e) + b_e )              (depthwise conv on W)
    q,k,v = y @ w{q,k,v}.T + b{q,k,v}                     ([rows, P])
    Q,K,V = fft(q),fft(k),fft(v)
    Wt = K * conj(Q) / sqrt(P) * V
    out = ifft(Wt).real @ wo.T + bo

Everything except the elementwise complex triple product is linear in x, so
the conv, the FFT, and the iFFT fold into host-precomputed projection
weights; the whole network collapses into three matmul stages and two
elementwise stages (see _fold_layer).  Weights are repacked host-side into
a handful of large consumption-ordered DRAM tensors so the kernel issues
few, big DMAs that land exactly in matmul order (many small DMAs serialize
on the shared HWDGE completion-semaphore lanes and starve the PE).
"""
import sys

import numpy as np

sys.path.insert(0, "/opt/trn_rl_repo")

import concourse.bass as bass
import concourse.mybir as mybir
import concourse.tile as tile
from concourse import bacc
from concourse.bass_utils import run_bass_kernel_spmd

BS, CNT, W, P, E, KK, L = 32, 128, 2048, 1024, 8, 25, 2
H = P // 2                    # 512 slots per packed block
NCORES = 8
ROWS = (BS // NCORES) * CNT   # 512 rows per core
KT = W // 128                 # 16 contraction tiles (stage 1)
MT = (3 * P) // 128           # 24 output tiles (stages 1,2: q|k|v packed)
ST = P // 128                 # 8 contraction tiles (stages 2,3)
WT = W // 128                 # 16 output tiles (stage 3)
F32 = mybir.dt.float32
ACT = mybir.dt.float16
ACT_NP = np.float16
IDENT = mybir.ActivationFunctionType.Identity


def _fold_layer(conv_w, conv_b, wq, bq, wk, bk, wv, bv, wo, bo):
    """Fold conv + FFT into projection weights (float64 math).

    Returns EW [W, 3*P], Sbias [3*P], WoP [P, W] (rows interleaved
    [A0 B0 A1 B1 A2 B2 A3 B3] by 128-tile), bo [W].
    """
    f64 = np.float64
    wbar = conv_w.astype(f64).mean(axis=0)[0]          # [KK]
    bbar = conv_b.astype(f64).mean()

    idx = np.arange(W)
    D = idx[None, :] - idx[:, None] + (KK // 2)        # C[w,u] = wbar[u-w+12]
    C = np.where((D >= 0) & (D < KK), wbar[np.clip(D, 0, KK - 1)], 0.0)

    def pack_fwd(wmat, bvec, scale=1.0):
        Wf = np.fft.fft(wmat.astype(f64), axis=0)      # [P, W]
        Bf = np.fft.fft(bvec.astype(f64))              # [P]
        cols = np.empty((W, P), dtype=f64)
        cols[:, :H] = Wf[:H, :].real.T
        cols[:, H] = Wf[H, :].real
        cols[:, H + 1:] = Wf[1:H, :].imag.T
        bias = np.empty(P, dtype=f64)
        bias[:H] = Bf[:H].real
        bias[H] = Bf[H].real
        bias[H + 1:] = Bf[1:H].imag
        return cols * scale, bias * scale

    s = 1.0 / np.sqrt(f64(P))
    cq, bq_p = pack_fwd(wq, bq)
    ck, bk_p = pack_fwd(wk, bk)
    cv, bv_p = pack_fwd(wv, bv, scale=s)
    cols = np.concatenate([cq, ck, cv], axis=1)        # [W, 3P]
    bias = np.concatenate([bq_p, bk_p, bv_p])

    EW = C.T @ cols
    Sbias = bbar * cols.sum(axis=0) + bias

    G = np.fft.ifft(wo.astype(f64), axis=1)            # [W, P]
    WoP = np.empty((P, W), dtype=f64)
    WoP[0] = G[:, 0].real
    WoP[1:H] = 2.0 * G[:, 1:H].real.T
    WoP[H] = G[:, H].real
    WoP[H + 1:] = -2.0 * G[:, 1:H].imag.T
    # interleave row-tiles A0 B0 A1 B1 ... to match Wcat production order
    WoP = WoP.reshape(2, 4, 128, W).transpose(1, 0, 2, 3).reshape(P, W)

    return EW, Sbias, WoP, bo.astype(f64)


def _build_module():
    nc = bacc.Bacc("TRN2", target_bir_lowering=False, debug=False)

    # x: one [128, KT*ROWS] tensor; column block k = contraction k-tile.
    xin = nc.dram_tensor("xin", [128, KT * ROWS], ACT, kind="ExternalInput")
    # stage-1 t=0 weights: all six chains k-interleaved: col (k*6+b)*128
    ewt0 = nc.dram_tensor("ewt0", [128, 6 * W], ACT, kind="ExternalInput")
    # stage-1 groups t=1..3: 6 chain-tiles each, chain-major
    ew1g = nc.dram_tensor("ew1g", [3, 128, 6 * W], ACT, kind="ExternalInput")
    # stage-2 groups t=0..3: 6 chain-tiles each, chain-major
    m12g = nc.dram_tensor("m12g", [4, 128, 6 * P], ACT, kind="ExternalInput")
    # stage-3 weights in quarters of 4 output tiles
    wop2q = nc.dram_tensor("wop2q", [4, 128, 4 * P], ACT,
                           kind="ExternalInput")
    # [sb1(24) | sb2(24) | bo2(16)] packed into one [128, 64] tensor
    biases = nc.dram_tensor("biases", [128, 2 * MT + WT], F32,
                            kind="ExternalInput")
    xout = nc.dram_tensor("xout", [WT, 128, ROWS], ACT, kind="ExternalOutput")

    with tile.TileContext(nc) as tc:
        with (
            tc.tile_pool(name="xbuf", bufs=1) as xpool,
            tc.tile_pool(name="bias", bufs=1) as bpool,
            tc.tile_pool(name="warm", bufs=1) as warmpool,
            tc.tile_pool(name="wt0", bufs=1) as wt0pool,
            tc.tile_pool(name="wfg", bufs=2) as wfgpool,
            tc.tile_pool(name="wm", bufs=2) as wmpool,
            tc.tile_pool(name="wo2", bufs=4) as wo2pool,
            tc.tile_pool(name="spec", bufs=16) as spool,
            tc.tile_pool(name="wt", bufs=16) as wtpool,
            tc.tile_pool(name="ew", bufs=8) as ewpool,
            tc.tile_pool(name="out", bufs=4) as opool,
            tc.tile_pool(name="psum", bufs=8, space="PSUM") as pspool,
        ):
            # ---- PE warm-up: the HAM clock gate holds the PE at 1.2 GHz
            # until it has seen ~3.4us of gap-free activity.  These dummies
            # run while the first DMAs are in flight.  The early phase is
            # HBM-bandwidth-bound (x + the t=0/t=1 weight groups total
            # ~8.2MB against ~330GB/s from ring-up at ~8.2us), so the
            # dummy count is sized to push the first real matmul to
            # ~12.4us: delivery then stays ahead of the 246GB/s consumption
            # for the whole t=0 sweep, which removes ~3.6us of mid-stream
            # stalls at the cost of ~2.1us of extra dummies.
            wdummy = warmpool.tile([128, 256], ACT, tag="warm")
            nc.vector.memset(wdummy[:], 0.0)
            psd = pspool.tile([128, 256], F32, tag="ps", name="psd")
            for _ in range(34):
                nc.tensor.matmul(psd[:], wdummy[:, 0:128], wdummy[:],
                                 start=True, stop=True)

            # ---- input streaming: few, large, need-ordered DMAs.
            # sync carries the t=0 weight mega-tile (k-order chunks so the
            # first matmul waits only for one 192KB chunk), scalar carries x.
            wt0 = wt0pool.tile([128, 6 * W], ACT, tag="wt0")
            xb = xpool.tile([128, KT * ROWS], ACT, tag="x")
            for lo, hi in ((0, 768), (768, 1536), (1536, 3072),
                           (3072, 6144), (6144, 9216), (9216, 12288)):
                nc.sync.dma_start(wt0[:, lo:hi], ewt0[:, lo:hi])

            for lo, hi in ((0, 512), (512, 1024), (1024, 1536),
                           (1536, 2048), (2048, 3072), (3072, 4096),
                           (4096, 6144), (6144, 8192)):
                nc.scalar.dma_start(xb[:, lo:hi], xin[:, lo:hi])

            btile = bpool.tile([128, 2 * MT + WT], F32, tag="biases")
            nc.sync.dma_start(btile[:], biases[:])

            def xslice(k):
                return xb[:, bass.ts(k, ROWS)]

            def sb1_col(j):
                return btile[:, j:j + 1]

            def sb2_col(j):
                return btile[:, MT + j:MT + j + 1]

            def bo2_col(j):
                return btile[:, 2 * MT + j:2 * MT + j + 1]

            def elementwise(St, first):
                """complex triple product on one partition-row group.

                All operands fp16: contiguous 16-bit tensor_tensor hits the
                DVE's 2x packed mode, halving vector time.
                """
                qA, qB, kA, kB, vA, vB = St
                v = nc.vector
                cr = ewpool.tile([128, ROWS], ACT, tag="ew")
                ci = ewpool.tile([128, ROWS], ACT, tag="ew")
                t0 = ewpool.tile([128, ROWS], ACT, tag="ew")
                v.tensor_mul(cr[:], kA[:], qA[:])
                v.tensor_mul(t0[:], kB[:], qB[:])
                v.tensor_add(cr[:], cr[:], t0[:])
                v.tensor_mul(ci[:], kB[:], qA[:])
                v.tensor_mul(t0[:], kA[:], qB[:])
                v.tensor_sub(ci[:], ci[:], t0[:])
                wr = wtpool.tile([128, ROWS], ACT, tag="wt")
                wi = wtpool.tile([128, ROWS], ACT, tag="wt")
                v.tensor_mul(wr[:], cr[:], vA[:])
                v.tensor_mul(t0[:], ci[:], vB[:])
                v.tensor_sub(wr[:], wr[:], t0[:])
                v.tensor_mul(wi[:], cr[:], vB[:])
                v.tensor_mul(t0[:], ci[:], vA[:])
                v.tensor_add(wi[:], wi[:], t0[:])
                if first:
                    # slot 0: A holds DC, B holds Nyquist — both real
                    v.tensor_mul(t0[0:1, :], qA[0:1, :], kA[0:1, :])
                    v.tensor_mul(wr[0:1, :], t0[0:1, :], vA[0:1, :])
                    v.tensor_mul(t0[0:1, :], qB[0:1, :], kB[0:1, :])
                    v.tensor_mul(wi[0:1, :], t0[0:1, :], vB[0:1, :])
                return wr, wi

            # ---- stage 1: S1 = x @ EW1 + b1, pipelined elementwise ----
            # t=0: chains b=0..3 are k-interleaved in both issue order and
            # the weight layout, so weights and x arrive exactly in matmul
            # order while the PE is still cold.
            Wcat1 = [None] * ST
            wfg_tiles = {}
            for t in range(4):
                if t < 3:                       # prefetch group t+1 weights
                    wfgt = wfgpool.tile([128, 6 * W], ACT, tag="wfg",
                                        name="wfg")
                    nc.sync.dma_start(wfgt[:, 0:6144], ew1g[t][:, 0:6144])
                    nc.sync.dma_start(wfgt[:, 6144:], ew1g[t][:, 6144:])
                    wfg_tiles[t + 1] = wfgt
                if t == 3:                      # prefetch first m12 group
                    wm0 = wmpool.tile([128, 6 * P], ACT, tag="wm", name="wm")
                    nc.sync.dma_start(wm0[:, 0:3072], m12g[0][:, 0:3072])
                    nc.sync.dma_start(wm0[:, 3072:], m12g[0][:, 3072:])
                    wfg_tiles["m0"] = wm0

                St = []
                if t == 0:
                    # all six chains k-interleaved: weights and x are
                    # consumed in exactly DMA-arrival order, and the tiny
                    # data-independent fillers keep the HAM activity window
                    # busy through any arrival jitter while the PE warms.
                    chains = [pspool.tile([128, ROWS], F32, tag="ps",
                                          name="ps") for _ in range(6)]
                    for k in range(KT):
                        for b in range(6):
                            nc.tensor.matmul(
                                chains[b][:],
                                wt0[:, bass.ts(k * 6 + b, 128)], xslice(k),
                                start=(k == 0), stop=(k == KT - 1))
                    for b in range(6):
                        Sj = spool.tile([128, ROWS], ACT, tag="spec")
                        nc.scalar.activation(Sj[:], chains[b][:], IDENT,
                                             bias=sb1_col(b * 4))
                        St.append(Sj)
                else:
                    wfgt = wfg_tiles[t]
                    for b in range(6):
                        j = b * 4 + t
                        ps = pspool.tile([128, ROWS], F32, tag="ps")
                        for k in range(KT):
                            nc.tensor.matmul(
                                ps[:], wfgt[:, b * W:][:, bass.ts(k, 128)],
                                xslice(k),
                                start=(k == 0), stop=(k == KT - 1))
                        Sj = spool.tile([128, ROWS], ACT, tag="spec")
                        nc.scalar.activation(Sj[:], ps[:], IDENT,
                                             bias=sb1_col(j))
                        St.append(Sj)
                wr, wi = elementwise(St, t == 0)
                Wcat1[2 * t] = wr
                Wcat1[2 * t + 1] = wi

            # ---- stage 2: S2 = Wt1 @ M12 + b2, pipelined elementwise ----
            # m12 group t+1 and the wop2 quarters prefetch during group t.
            Wcat2 = [None] * ST
            wo2_tiles = []
            for t in range(4):
                wmt = wfg_tiles.pop("m0") if t == 0 else wfg_tiles[("m", t)]
                if t < 3:
                    wmn = wmpool.tile([128, 6 * P], ACT, tag="wm", name="wm")
                    nc.sync.dma_start(wmn[:, 0:3072], m12g[t + 1][:, 0:3072])
                    nc.sync.dma_start(wmn[:, 3072:], m12g[t + 1][:, 3072:])
                    wfg_tiles[("m", t + 1)] = wmn
                if t < 4:                       # wop2 quarter prefetch
                    w2 = wo2pool.tile([128, 4 * P], ACT, tag="wo2",
                                      name="wo2")
                    nc.scalar.dma_start(w2[:], wop2q[t])
                    wo2_tiles.append(w2)

                St = []
                for b in range(6):
                    j = b * 4 + t
                    ps = pspool.tile([128, ROWS], F32, tag="ps")
                    for s in range(ST):
                        nc.tensor.matmul(
                            ps[:], wmt[:, b * P:][:, bass.ts(s, 128)],
                            Wcat1[s][:],
                            start=(s == 0), stop=(s == ST - 1))
                    Sj = spool.tile([128, ROWS], ACT, tag="spec")
                    nc.scalar.activation(Sj[:], ps[:], IDENT,
                                         bias=sb2_col(j))
                    St.append(Sj)
                wr, wi = elementwise(St, t == 0)
                Wcat2[2 * t] = wr
                Wcat2[2 * t + 1] = wi

            # ---- stage 3: out = Wt2 @ WoP2 + bo2 ----
            # evac+bias on the scalar engine; fp16 output; the final tile is
            # split in half so its evac/DMA tail is shorter.
            # first two tiles contract s=0..5 first (ready since mid-stage
            # 2) and finish with s=6,7 once the last elementwise group
            # lands, so the PE rolls through the 2->3 boundary stall-free
            wave = []
            for j in (0, 1):
                wcol = wo2_tiles[0][:, j * P:]
                ps = pspool.tile([128, ROWS], F32, tag="ps", name="ps")
                for s in range(ST - 2):
                    nc.tensor.matmul(
                        ps[:], wcol[:, bass.ts(s, 128)], Wcat2[s][:],
                        start=(s == 0), stop=False)
                wave.append((j, wcol, ps))
            for j, wcol, ps in wave:
                for s in (ST - 2, ST - 1):
                    nc.tensor.matmul(
                        ps[:], wcol[:, bass.ts(s, 128)], Wcat2[s][:],
                        start=False, stop=(s == ST - 1))
                ostage = opool.tile([128, ROWS], ACT, tag="out")
                nc.scalar.activation(ostage[:], ps[:], IDENT,
                                     bias=bo2_col(j))
                (nc.scalar if j % 2 else nc.sync).dma_start(
                    xout[j], ostage[:])
            for j in range(2, WT - 1):
                w2 = wo2_tiles[j // 4]
                wcol = w2[:, (j % 4) * P:]
                ps = pspool.tile([128, ROWS], F32, tag="ps")
                for s in range(ST):
                    nc.tensor.matmul(
                        ps[:], wcol[:, bass.ts(s, 128)], Wcat2[s][:],
                        start=(s == 0), stop=(s == ST - 1))
                ostage = opool.tile([128, ROWS], ACT, tag="out")
                nc.scalar.activation(ostage[:], ps[:], IDENT,
                                     bias=bo2_col(j))
                (nc.scalar if j % 2 else nc.sync).dma_start(
                    xout[j], ostage[:])

            # final tile: two half-row PSUM chains in separate banks so
            # the evacs run on scalar AND vector in parallel and both
            # output DMAs overlap — shortens the kernel-tail drain.
            jl = WT - 1
            wcol = wo2_tiles[jl // 4][:, (jl % 4) * P:]
            ps_a = pspool.tile([128, ROWS // 2], F32, tag="ps", name="psa")
            ps_b = pspool.tile([128, ROWS // 2], F32, tag="ps", name="psb")
            for s in range(ST):
                nc.tensor.matmul(
                    ps_a[:], wcol[:, bass.ts(s, 128)],
                    Wcat2[s][:, 0:ROWS // 2],
                    start=(s == 0), stop=(s == ST - 1))
            for s in range(ST):
                nc.tensor.matmul(
                    ps_b[:], wcol[:, bass.ts(s, 128)],
                    Wcat2[s][:, ROWS // 2:],
                    start=(s == 0), stop=(s == ST - 1))
            oa = opool.tile([128, ROWS // 2], ACT, tag="outh")
            nc.scalar.activation(oa[:], ps_a[:], IDENT, bias=bo2_col(jl))
            nc.sync.dma_start(xout[jl][:, 0:ROWS // 2], oa[:])
            ob = opool.tile([128, ROWS // 2], ACT, tag="outh")
            nc.vector.tensor_scalar_add(ob[:], ps_b[:], bo2_col(jl))
            nc.scalar.dma_start(xout[jl][:, ROWS // 2:], ob[:])
    nc.compile()
    return nc


_MODULE_CACHE = {}


def _get_module():
    if "nc" not in _MODULE_CACHE:
        _MODULE_CACHE["nc"] = _build_module()
    return _MODULE_CACHE["nc"]


def _prepare_weight_maps(conv_w, conv_b, wq, bq, wk, bk, wv, bv, wo, bo):
    folds = [_fold_layer(conv_w[l], conv_b[l], wq[l], bq[l], wk[l], bk[l],
                         wv[l], bv[l], wo[l], bo[l]) for l in range(L)]
    EW1, Sb1, WoP1, _bo1 = folds[0]
    EW2, Sb2, WoP2, bo2 = folds[1]
    M12 = WoP1 @ EW2                               # [P, 3P], fp64
    Sb2e = _bo1 @ EW2 + Sb2                        # [3P]

    def pack(Wm, n_k, n_m):
        # [n_k*128, n_m*128] -> [n_m, 128, n_k*128] partition-contiguous
        return np.ascontiguousarray(
            Wm.reshape(n_k, 128, n_m, 128).transpose(2, 1, 0, 3)
            .reshape(n_m, 128, n_k * 128).astype(ACT_NP))

    ew1 = pack(EW1, KT, MT)                        # [24, 128, 2048]
    m12 = pack(M12, ST, MT)                        # [24, 128, 1024]
    wop2 = pack(WoP2, ST, WT)                      # [16, 128, 1024]

    # t=0 mega-tile: all six chains k-interleaved, col ((k*6+b)*128 + m)
    ewt0 = (np.stack([ew1[b * 4] for b in range(6)])      # [6,128,2048]
            .reshape(6, 128, KT, 128).transpose(1, 2, 0, 3)
            .reshape(128, 6 * W))

    ew1g = np.stack([
        np.concatenate([ew1[b * 4 + t] for b in range(6)], axis=1)
        for t in (1, 2, 3)])
    m12gv = np.stack([
        np.concatenate([m12[b * 4 + t] for b in range(6)], axis=1)
        for t in range(4)])
    wop2qv = np.stack([
        np.concatenate([wop2[q * 4 + j] for j in range(4)], axis=1)
        for q in range(4)])

    biases = np.concatenate([
        Sb1.reshape(MT, 128).T, Sb2e.reshape(MT, 128).T,
        bo2.reshape(WT, 128).T], axis=1).astype(np.float32)
    return {
        "ewt0": np.ascontiguousarray(ewt0),
        "ew1g": np.ascontiguousarray(ew1g),
        "m12g": np.ascontiguousarray(m12gv),
        "wop2q": np.ascontiguousarray(wop2qv),
        "biases": np.ascontiguousarray(biases),
    }


def _make_in_maps(inputs):
    x = np.asarray(inputs["x"], dtype=np.float32)
    wmap = _prepare_weight_maps(
        np.asarray(inputs["conv_w"]), np.asarray(inputs["conv_b"]),
        np.asarray(inputs["wq"]), np.asarray(inputs["bq"]),
        np.asarray(inputs["wk"]), np.asarray(inputs["bk"]),
        np.asarray(inputs["wv"]), np.asarray(inputs["bv"]),
        np.asarray(inputs["wo"]), np.asarray(inputs["bo"]))
    per_core = BS // NCORES
    in_maps = []
    for c in range(NCORES):
        xc = x[c * per_core:(c + 1) * per_core].reshape(ROWS, W)
        xin = np.ascontiguousarray(
            xc.reshape(ROWS, KT, 128).transpose(2, 1, 0)
            .reshape(128, KT * ROWS).astype(ACT_NP))
        in_maps.append({"xin": xin, **wmap})
    return in_maps


def kernel(x, conv_w, conv_b, wq, bq, wk, bk, wv, bv, wo, bo):
    in_maps = _make_in_maps(dict(
        x=x, conv_w=conv_w, conv_b=conv_b, wq=wq, bq=bq, wk=wk, bk=bk,
        wv=wv, bv=bv, wo=wo, bo=bo))
    nc = _get_module()
    res = run_bass_kernel_spmd(nc, in_maps, list(range(NCORES)))

    per_core = BS // NCORES
    outs = []
    for c in range(NCORES):
        xo = res.results[c]["xout"]                    # [WT, 128, ROWS]
        outs.append(xo.astype(np.float32)
                    .transpose(2, 0, 1).reshape(per_core, CNT, W))
    return np.concatenate(outs, axis=0).astype(np.float32)

